# revision 3
# baseline (speedup 1.0000x reference)
"""Trainium2 Bass kernel for nn_GatedBlock (moe_routing).

Math (reference collapses): the (NB,BS,BS) reshape of weight maps block k to
rows [128k, 128k+128) of weight, so
    out[b, i] = g[b, i // 128] * (x @ W.T)[b, i] + bias[i]
with g = sigmoid(x @ gate_w + gate_b), bottom-8 of 16 gates zeroed per row.

Sharding: output-dim (i) split 8 ways -> 256 rows of W (= 2 gate blocks) per
core.  Each core receives:
  xt  (128, KT, 32)   x.T in k-tile-major layout (replicated)
  rhs (128, KT, 272)  [W_shard.T | gate_w[:, perm]] k-tile-major; perm puts
                      this core's two gate columns at positions 0,1 so the
                      program is SPMD-uniform
  epi (32, 272)       [bias_shard bcast | gate_b[perm] bcast]
One PSUM accumulation over KT k-tiles computes both the 256-col main matmul
and the 16-col gate linear.  Top-8 gate mask via vector.max + match_replace.
"""

import sys

for _p in ("/opt/trn_rl_repo", "/root/.axon_site/_ro/trn_rl_repo"):
    if _p not in sys.path:
        sys.path.append(_p)

import numpy as np

B = 32          # batch
D = 2048        # model dim
NB = 16         # gate blocks
BLK = D // NB   # 128 output rows per gate block
N_CORES = 8
NOUT = D // N_CORES       # 256 output cols per core
KT = D // 128             # 16 k-tiles
NN = NOUT + NB            # 272 = matmul free dim (main + gate cols)

MODE = "f32"              # "f32" | "f32r" | "bf16x2"
DMA_GROUP = 2             # k-tiles per rhs DMA

_compiled = {}


def _build(mode):
    import concourse.bacc as bacc
    import concourse.tile as tile
    import concourse.mybir as mybir

    f32 = mybir.dt.float32
    if mode == "f32":
        mm_dt, n_split = f32, 1
    elif mode == "f32r":
        mm_dt, n_split = mybir.dt.float32r, 1
    elif mode == "bf16x2":
        mm_dt, n_split = mybir.dt.bfloat16, 2
    else:
        raise ValueError(mode)

    nc = bacc.Bacc("TRN2", target_bir_lowering=False, debug=False,
                   num_devices=N_CORES)

    xt_d = [nc.dram_tensor(f"xt{s}", [128, KT, B], mm_dt, kind="ExternalInput")
            for s in range(n_split)]
    rhs_d = [nc.dram_tensor(f"rhs{s}", [128, KT, NN], mm_dt, kind="ExternalInput")
             for s in range(n_split)]
    epi_d = nc.dram_tensor("epi", [B, NN], f32, kind="ExternalInput")
    out_d = nc.dram_tensor("out", [B, NOUT], f32, kind="ExternalOutput")

    with tile.TileContext(nc) as tc:
        with (
            tc.tile_pool(name="sb", bufs=1) as sb,
            tc.tile_pool(name="ps", bufs=1, space="PSUM") as psp,
        ):
            xt = [sb.tile([128, KT, B], mm_dt, name=f"xt_sb{s}", tag=f"xt_sb{s}")
                  for s in range(n_split)]
            rhs = [sb.tile([128, KT, NN], mm_dt, name=f"rhs_sb{s}", tag=f"rhs_sb{s}")
                   for s in range(n_split)]
            epi = sb.tile([B, NN], f32, name="epi_sb", tag="epi_sb")
            graw = sb.tile([B, NB], f32, name="graw", tag="graw")
            g = sb.tile([B, NB], f32, name="g", tag="g")
            m8 = sb.tile([B, 8], f32, name="m8", tag="m8")
            rep = sb.tile([B, NB], f32, name="rep", tag="rep")
            gk = sb.tile([B, NB], f32, name="gk", tag="gk")
            outt = sb.tile([B, NOUT], f32, name="outt", tag="outt")
            ps = psp.tile([B, NN], f32, name="ps", tag="ps")

            for s in range(n_split):
                nc.sync.dma_start(xt[s][:], xt_d[s].ap())
            nc.sync.dma_start(epi[:], epi_d.ap())
            for s in range(n_split):
                for t0 in range(0, KT, DMA_GROUP):
                    nc.sync.dma_start(
                        rhs[s][:, t0:t0 + DMA_GROUP, :],
                        rhs_d[s].ap()[:, t0:t0 + DMA_GROUP, :],
                    )

            # accumulation passes: f32/f32r -> [(0,0)]
            # bf16x2 -> hi*hi, hi*lo, lo*hi
            passes = [(0, 0)] if n_split == 1 else [(0, 0), (0, 1), (1, 0)]
            n_mm = len(passes) * KT
            i = 0
            for (sx, sw) in passes:
                for t in range(KT):
                    nc.tensor.matmul(
                        ps[:], xt[sx][:, t, :], rhs[sw][:, t, :],
                        start=(i == 0), stop=(i == n_mm - 1),
                    )
                    i += 1

            # gates: sigmoid(glin + gate_b), keep top-8 of 16
            nc.vector.tensor_add(graw[:], ps[:, NOUT:NN], epi[:, NOUT:NN])
            nc.scalar.activation(g[:], graw[:],
                                 mybir.ActivationFunctionType.Sigmoid)
            nc.vector.max(m8[:], g[:])
            nc.vector.match_replace(rep[:], m8[:], g[:], 0.0)
            nc.vector.tensor_sub(gk[:], g[:], rep[:])

            # out = psum * g[block] + bias, per 128-col block
            for h in range(NOUT // BLK):
                sl = slice(h * BLK, (h + 1) * BLK)
                nc.vector.scalar_tensor_tensor(
                    outt[:, sl], ps[:, sl], gk[:, h:h + 1], epi[:, sl],
                    mybir.AluOpType.mult, mybir.AluOpType.add,
                )

            nc.sync.dma_start(out_d.ap(), outt[:])

    nc.compile()
    return nc


def _tile_major(a):
    """(D, n) -> (128, KT, n) k-tile-major contiguous."""
    n = a.shape[1]
    return np.ascontiguousarray(a.reshape(KT, 128, n).transpose(1, 0, 2))


def _split_parts(a, mode):
    """Split fp32 array into matmul-dtype parts per MODE."""
    if mode == "f32" or mode == "f32r":
        return [np.ascontiguousarray(a, dtype=np.float32)]
    import ml_dtypes
    hi = a.astype(ml_dtypes.bfloat16)
    lo = (a - hi.astype(np.float32)).astype(ml_dtypes.bfloat16)
    return [hi, lo]


def build_in_maps(x, gate_w, gate_b, weight, bias):
    x = np.asarray(x, dtype=np.float32)
    gate_w = np.asarray(gate_w, dtype=np.float32)
    gate_b = np.asarray(gate_b, dtype=np.float32)
    weight = np.asarray(weight, dtype=np.float32)
    bias = np.asarray(bias, dtype=np.float32)

    xt_parts = [_tile_major(p) for p in _split_parts(x.T, MODE)]

    in_maps = []
    for c in range(N_CORES):
        perm = [2 * c, 2 * c + 1] + [k for k in range(NB)
                                     if k not in (2 * c, 2 * c + 1)]
        w_shard = weight[c * NOUT:(c + 1) * NOUT, :]          # (256, 2048)
        rhs = np.concatenate([w_shard.T, gate_w[:, perm]], axis=1)  # (2048, 272)
        rhs_parts = [_tile_major(p) for p in _split_parts(rhs, MODE)]
        epi = np.concatenate([
            np.broadcast_to(bias[c * NOUT:(c + 1) * NOUT], (B, NOUT)),
            np.broadcast_to(gate_b[perm], (B, NB)),
        ], axis=1).astype(np.float32)
        m = {"epi": np.ascontiguousarray(epi)}
        for s, (xp, rp) in enumerate(zip(xt_parts, rhs_parts)):
            m[f"xt{s}"] = xp
            m[f"rhs{s}"] = rp
        in_maps.append(m)
    return in_maps


def kernel(x, gate_w, gate_b, weight, bias):
    from concourse.bass_utils import run_bass_kernel_spmd

    if MODE not in _compiled:
        _compiled[MODE] = _build(MODE)
    nc = _compiled[MODE]

    in_maps = build_in_maps(x, gate_w, gate_b, weight, bias)
    res = run_bass_kernel_spmd(nc, in_maps, list(range(N_CORES)))
    out = np.concatenate([res.results[c]["out"] for c in range(N_CORES)], axis=1)
    return out.astype(np.float32)


# revision 4
# speedup vs baseline: 1.1611x; 1.1611x over previous
"""Trainium2 Bass kernel for nn_GatedBlock (moe_routing).

Math (reference collapses): the (NB,BS,BS) reshape of weight maps block k to
rows [128k, 128k+128) of weight, so
    out[b, i] = g[b, i // 128] * (x @ W.T)[b, i] + bias[i]
with g = sigmoid(x @ gate_w + gate_b), bottom-8 of 16 gates zeroed per row.

Sharding: output-dim (i) split 8 ways -> 256 rows of W (= 2 gate blocks) per
core.  Each core receives:
  xt  (128, KT, 32)   x.T in k-tile-major layout (replicated)
  rhs (128, KT, 272)  [W_shard.T | gate_w[:, perm]] k-tile-major; perm puts
                      this core's two gate columns at positions 0,1 so the
                      program is SPMD-uniform
  epi (32, 272)       [bias_shard bcast | gate_b[perm] bcast]
One PSUM accumulation over KT k-tiles computes both the 256-col main matmul
and the 16-col gate linear.  Top-8 gate mask via vector.max + match_replace.
"""

import sys

for _p in ("/opt/trn_rl_repo", "/root/.axon_site/_ro/trn_rl_repo"):
    if _p not in sys.path:
        sys.path.append(_p)

import numpy as np

B = 32          # batch
D = 2048        # model dim
NB = 16         # gate blocks
BLK = D // NB   # 128 output rows per gate block
N_CORES = 8
NOUT = D // N_CORES       # 256 output cols per core
KT = D // 128             # 16 k-tiles
NN = NOUT + NB            # 272 = matmul free dim (main + gate cols)

import os as _os
MODE = _os.environ.get("GATED_MODE", "f32")   # "f32" | "f32r" | "bf16x2"
DMA_GROUP = int(_os.environ.get("GATED_DMA_GROUP", "2"))  # k-tiles per rhs DMA

_compiled = {}


def _build(mode):
    import concourse.bacc as bacc
    import concourse.tile as tile
    import concourse.mybir as mybir

    f32 = mybir.dt.float32
    if mode == "f32":
        mm_dt, n_split = f32, 1
    elif mode == "f32r":
        mm_dt, n_split = mybir.dt.float32r, 1
    elif mode == "bf16x2":
        mm_dt, n_split = mybir.dt.bfloat16, 2
    else:
        raise ValueError(mode)

    nc = bacc.Bacc("TRN2", target_bir_lowering=False, debug=False,
                   num_devices=N_CORES)

    xt_d = [nc.dram_tensor(f"xt{s}", [128, KT, B], mm_dt, kind="ExternalInput")
            for s in range(n_split)]
    rhs_d = [nc.dram_tensor(f"rhs{s}", [128, KT, NN], mm_dt, kind="ExternalInput")
             for s in range(n_split)]
    epi_d = nc.dram_tensor("epi", [B, NN], f32, kind="ExternalInput")
    out_d = nc.dram_tensor("out", [B, NOUT], f32, kind="ExternalOutput")

    with tile.TileContext(nc) as tc:
        with (
            tc.tile_pool(name="sb", bufs=1) as sb,
            tc.tile_pool(name="ps", bufs=1, space="PSUM") as psp,
        ):
            xt = [sb.tile([128, KT, B], mm_dt, name=f"xt_sb{s}", tag=f"xt_sb{s}")
                  for s in range(n_split)]
            rhs = [sb.tile([128, KT, NN], mm_dt, name=f"rhs_sb{s}", tag=f"rhs_sb{s}")
                   for s in range(n_split)]
            epi = sb.tile([B, NN], f32, name="epi_sb", tag="epi_sb")
            graw = sb.tile([B, NB], f32, name="graw", tag="graw")
            g = sb.tile([B, NB], f32, name="g", tag="g")
            m8 = sb.tile([B, 8], f32, name="m8", tag="m8")
            rep = sb.tile([B, NB], f32, name="rep", tag="rep")
            gk = sb.tile([B, NB], f32, name="gk", tag="gk")
            outt = sb.tile([B, NOUT], f32, name="outt", tag="outt")
            ps = psp.tile([B, NN], f32, name="ps", tag="ps")

            for s in range(n_split):
                nc.sync.dma_start(xt[s][:], xt_d[s].ap())
            nc.sync.dma_start(epi[:], epi_d.ap())
            for s in range(n_split):
                for t0 in range(0, KT, DMA_GROUP):
                    nc.sync.dma_start(
                        rhs[s][:, t0:t0 + DMA_GROUP, :],
                        rhs_d[s].ap()[:, t0:t0 + DMA_GROUP, :],
                    )

            # accumulation passes: f32/f32r -> [(0,0)]
            # bf16x2 -> hi*hi, hi*lo, lo*hi
            passes = [(0, 0)] if n_split == 1 else [(0, 0), (0, 1), (1, 0)]
            n_mm = len(passes) * KT
            i = 0
            for (sx, sw) in passes:
                for t in range(KT):
                    nc.tensor.matmul(
                        ps[:], xt[sx][:, t, :], rhs[sw][:, t, :],
                        start=(i == 0), stop=(i == n_mm - 1),
                    )
                    i += 1

            # gates: sigmoid(glin + gate_b), keep top-8 of 16
            nc.vector.tensor_add(graw[:], ps[:, NOUT:NN], epi[:, NOUT:NN])
            nc.scalar.activation(g[:], graw[:],
                                 mybir.ActivationFunctionType.Sigmoid)
            nc.vector.max(m8[:], g[:])
            nc.vector.match_replace(rep[:], m8[:], g[:], 0.0)
            nc.vector.tensor_sub(gk[:], g[:], rep[:])

            # out = psum * g[block] + bias, per 128-col block
            for h in range(NOUT // BLK):
                sl = slice(h * BLK, (h + 1) * BLK)
                nc.vector.scalar_tensor_tensor(
                    outt[:, sl], ps[:, sl], gk[:, h:h + 1], epi[:, sl],
                    mybir.AluOpType.mult, mybir.AluOpType.add,
                )

            nc.sync.dma_start(out_d.ap(), outt[:])

    nc.compile()
    return nc


def _tile_major(a):
    """(D, n) -> (128, KT, n) k-tile-major contiguous."""
    n = a.shape[1]
    return np.ascontiguousarray(a.reshape(KT, 128, n).transpose(1, 0, 2))


def _split_parts(a, mode):
    """Split fp32 array into matmul-dtype parts per MODE."""
    if mode == "f32" or mode == "f32r":
        return [np.ascontiguousarray(a, dtype=np.float32)]
    import ml_dtypes
    hi = a.astype(ml_dtypes.bfloat16)
    lo = (a - hi.astype(np.float32)).astype(ml_dtypes.bfloat16)
    return [hi, lo]


def build_in_maps(x, gate_w, gate_b, weight, bias):
    x = np.asarray(x, dtype=np.float32)
    gate_w = np.asarray(gate_w, dtype=np.float32)
    gate_b = np.asarray(gate_b, dtype=np.float32)
    weight = np.asarray(weight, dtype=np.float32)
    bias = np.asarray(bias, dtype=np.float32)

    xt_parts = [_tile_major(p) for p in _split_parts(x.T, MODE)]

    in_maps = []
    for c in range(N_CORES):
        perm = [2 * c, 2 * c + 1] + [k for k in range(NB)
                                     if k not in (2 * c, 2 * c + 1)]
        w_shard = weight[c * NOUT:(c + 1) * NOUT, :]          # (256, 2048)
        rhs = np.concatenate([w_shard.T, gate_w[:, perm]], axis=1)  # (2048, 272)
        rhs_parts = [_tile_major(p) for p in _split_parts(rhs, MODE)]
        epi = np.concatenate([
            np.broadcast_to(bias[c * NOUT:(c + 1) * NOUT], (B, NOUT)),
            np.broadcast_to(gate_b[perm], (B, NB)),
        ], axis=1).astype(np.float32)
        m = {"epi": np.ascontiguousarray(epi)}
        for s, (xp, rp) in enumerate(zip(xt_parts, rhs_parts)):
            m[f"xt{s}"] = xp
            m[f"rhs{s}"] = rp
        in_maps.append(m)
    return in_maps


def kernel(x, gate_w, gate_b, weight, bias):
    from concourse.bass_utils import run_bass_kernel_spmd

    if MODE not in _compiled:
        _compiled[MODE] = _build(MODE)
    nc = _compiled[MODE]

    in_maps = build_in_maps(x, gate_w, gate_b, weight, bias)
    res = run_bass_kernel_spmd(nc, in_maps, list(range(N_CORES)))
    out = np.concatenate([res.results[c]["out"] for c in range(N_CORES)], axis=1)
    return out.astype(np.float32)


# revision 5
# speedup vs baseline: 1.1678x; 1.0058x over previous
"""Trainium2 Bass kernel for nn_GatedBlock (moe_routing).

Math (reference collapses): the (NB,BS,BS) reshape of weight maps block k to
rows [128k, 128k+128) of weight, so
    out[b, i] = g[b, i // 128] * (x @ W.T)[b, i] + bias[i]
with g = sigmoid(x @ gate_w + gate_b), bottom-8 of 16 gates zeroed per row.

Sharding: output-dim (i) split 8 ways -> 256 rows of W (= 2 gate blocks) per
core.  Each core receives:
  pre (128, KT, 48)   [x.T | gate_w[:, perm]] k-tile-major (gate cols permuted
                      so this core's two blocks sit at positions 0,1 -> the
                      program stays SPMD-uniform)
  rhs (128, KT, 256)  W_shard.T k-tile-major
  epi (32, 272)       [bias_shard bcast | gate_b[perm] bcast]
Gate linear runs as 16 tiny matmuls off the early `pre` load so the whole
sigmoid/top-8 chain hides under the W DMA phase; the main PSUM accumulation
then only needs two fused (psum*gate + bias) ops and the output DMA at the
tail.  Top-8 mask via vector.max + match_replace.
"""

import sys

for _p in ("/opt/trn_rl_repo", "/root/.axon_site/_ro/trn_rl_repo"):
    if _p not in sys.path:
        sys.path.append(_p)

import os as _os

import numpy as np

B = 32          # batch
D = 2048        # model dim
NB = 16         # gate blocks
BLK = D // NB   # 128 output rows per gate block
N_CORES = 8
NOUT = D // N_CORES       # 256 output cols per core
KT = D // 128             # 16 k-tiles
NPRE = B + NB             # 48 = xT cols + gate cols in the early array

MODE = _os.environ.get("GATED_MODE", "f32r")     # "f32" | "f32r" | "bf16x2"
DMA_GROUP = int(_os.environ.get("GATED_DMA_GROUP", "4"))   # k-tiles per rhs DMA
SPLIT_ENG = _os.environ.get("GATED_SPLIT_ENG", "1") == "1"  # alternate sync/scalar

_compiled = {}


def _build(mode):
    import concourse.bacc as bacc
    import concourse.tile as tile
    import concourse.mybir as mybir

    f32 = mybir.dt.float32
    if mode == "f32":
        mm_dt, n_split = f32, 1
    elif mode == "f32r":
        mm_dt, n_split = mybir.dt.float32r, 1
    elif mode == "bf16x2":
        mm_dt, n_split = mybir.dt.bfloat16, 2
    else:
        raise ValueError(mode)

    nc = bacc.Bacc("TRN2", target_bir_lowering=False, debug=False,
                   num_devices=N_CORES)

    pre_d = [nc.dram_tensor(f"pre{s}", [128, KT, NPRE], mm_dt, kind="ExternalInput")
             for s in range(n_split)]
    rhs_d = [nc.dram_tensor(f"rhs{s}", [128, KT, NOUT], mm_dt, kind="ExternalInput")
             for s in range(n_split)]
    epi_d = nc.dram_tensor("epi", [B, NOUT + NB], f32, kind="ExternalInput")
    out_d = nc.dram_tensor("out", [B, NOUT], f32, kind="ExternalOutput")

    with tile.TileContext(nc) as tc:
        with (
            tc.tile_pool(name="sb", bufs=1) as sb,
            tc.tile_pool(name="ps", bufs=1, space="PSUM") as psp,
        ):
            pre = [sb.tile([128, KT, NPRE], mm_dt, name=f"pre_sb{s}", tag=f"pre_sb{s}")
                   for s in range(n_split)]
            rhs = [sb.tile([128, KT, NOUT], mm_dt, name=f"rhs_sb{s}", tag=f"rhs_sb{s}")
                   for s in range(n_split)]
            epi = sb.tile([B, NOUT + NB], f32, name="epi_sb", tag="epi_sb")
            graw = sb.tile([B, NB], f32, name="graw", tag="graw")
            g = sb.tile([B, NB], f32, name="g", tag="g")
            m8 = sb.tile([B, 8], f32, name="m8", tag="m8")
            rep = sb.tile([B, NB], f32, name="rep", tag="rep")
            gk = sb.tile([B, NB], f32, name="gk", tag="gk")
            outt = sb.tile([B, NOUT], f32, name="outt", tag="outt")
            ps_g = psp.tile([B, NB], f32, name="ps_g", tag="ps_g")
            ps_m = psp.tile([B, NOUT], f32, name="ps_m", tag="ps_m")

            # early loads: pre (xT+gate_w) on sync, epi on scalar
            for s in range(n_split):
                nc.sync.dma_start(pre[s][:], pre_d[s].ap())
            nc.scalar.dma_start(epi[:], epi_d.ap())

            # rhs groups, alternating HWDGE queues
            engs = [nc.sync, nc.scalar] if SPLIT_ENG else [nc.sync]
            di = 0
            for s in range(n_split):
                for t0 in range(0, KT, DMA_GROUP):
                    engs[di % len(engs)].dma_start(
                        rhs[s][:, t0:t0 + DMA_GROUP, :],
                        rhs_d[s].ap()[:, t0:t0 + DMA_GROUP, :],
                    )
                    di += 1

            # accumulation passes: f32/f32r -> [(0,0)]; bf16x2 -> hh, hl, lh
            passes = [(0, 0)] if n_split == 1 else [(0, 0), (0, 1), (1, 0)]
            n_mm = len(passes) * KT

            # gate linear: 16 tiny matmuls off the early load only
            i = 0
            for (sx, sw) in passes:
                for t in range(KT):
                    nc.tensor.matmul(
                        ps_g[:], pre[sx][:, t, :B], pre[sw][:, t, B:NPRE],
                        start=(i == 0), stop=(i == n_mm - 1),
                    )
                    i += 1

            # gate chain (hides under the rhs DMA phase)
            nc.vector.tensor_add(graw[:], ps_g[:], epi[:, NOUT:NOUT + NB])
            nc.scalar.activation(g[:], graw[:],
                                 mybir.ActivationFunctionType.Sigmoid)
            nc.vector.max(m8[:], g[:])
            nc.vector.match_replace(rep[:], m8[:], g[:], 0.0)
            nc.vector.tensor_sub(gk[:], g[:], rep[:])

            # main matmul accumulation
            i = 0
            for (sx, sw) in passes:
                for t in range(KT):
                    nc.tensor.matmul(
                        ps_m[:], pre[sx][:, t, :B], rhs[sw][:, t, :],
                        start=(i == 0), stop=(i == n_mm - 1),
                    )
                    i += 1

            # out = psum * g[block] + bias, per 128-col block
            for h in range(NOUT // BLK):
                sl = slice(h * BLK, (h + 1) * BLK)
                nc.vector.scalar_tensor_tensor(
                    outt[:, sl], ps_m[:, sl], gk[:, h:h + 1], epi[:, sl],
                    mybir.AluOpType.mult, mybir.AluOpType.add,
                )

            nc.sync.dma_start(out_d.ap(), outt[:])

    nc.compile()
    return nc


def _tile_major(a):
    """(D, n) -> (128, KT, n) k-tile-major contiguous."""
    n = a.shape[1]
    return np.ascontiguousarray(a.reshape(KT, 128, n).transpose(1, 0, 2))


def _split_parts(a, mode):
    """Split fp32 array into matmul-dtype parts per MODE."""
    if mode == "f32" or mode == "f32r":
        return [np.ascontiguousarray(a, dtype=np.float32)]
    import ml_dtypes
    hi = a.astype(ml_dtypes.bfloat16)
    lo = (a - hi.astype(np.float32)).astype(ml_dtypes.bfloat16)
    return [hi, lo]


def build_in_maps(x, gate_w, gate_b, weight, bias):
    x = np.asarray(x, dtype=np.float32)
    gate_w = np.asarray(gate_w, dtype=np.float32)
    gate_b = np.asarray(gate_b, dtype=np.float32)
    weight = np.asarray(weight, dtype=np.float32)
    bias = np.asarray(bias, dtype=np.float32)

    in_maps = []
    for c in range(N_CORES):
        perm = [2 * c, 2 * c + 1] + [k for k in range(NB)
                                     if k not in (2 * c, 2 * c + 1)]
        pre = np.concatenate([x.T, gate_w[:, perm]], axis=1)      # (2048, 48)
        pre_parts = [_tile_major(p) for p in _split_parts(pre, MODE)]
        w_shard = weight[c * NOUT:(c + 1) * NOUT, :]              # (256, 2048)
        rhs_parts = [_tile_major(p)
                     for p in _split_parts(np.ascontiguousarray(w_shard.T), MODE)]
        epi = np.concatenate([
            np.broadcast_to(bias[c * NOUT:(c + 1) * NOUT], (B, NOUT)),
            np.broadcast_to(gate_b[perm], (B, NB)),
        ], axis=1).astype(np.float32)
        m = {"epi": np.ascontiguousarray(epi)}
        for s, (pp, rp) in enumerate(zip(pre_parts, rhs_parts)):
            m[f"pre{s}"] = pp
            m[f"rhs{s}"] = rp
        in_maps.append(m)
    return in_maps


def kernel(x, gate_w, gate_b, weight, bias):
    from concourse.bass_utils import run_bass_kernel_spmd

    if MODE not in _compiled:
        _compiled[MODE] = _build(MODE)
    nc = _compiled[MODE]

    in_maps = build_in_maps(x, gate_w, gate_b, weight, bias)
    res = run_bass_kernel_spmd(nc, in_maps, list(range(N_CORES)))
    out = np.concatenate([res.results[c]["out"] for c in range(N_CORES)], axis=1)
    return out.astype(np.float32)
